# revision 1
# baseline (speedup 1.0000x reference)
"""Trainium2 Bass kernel for nn_ActorNetwork (GNN message passing), 8 NeuronCores.

Strategy
--------
Data-parallel over the 256 graphs: core c owns graphs [32c, 32c+32).

Algebraic restructure (validated vs reference to ~6e-7 rel err):
  * GCNConv aggregation is a dense per-graph matmul with the block-diagonal
    normalized adjacency A_hat = D^-1/2 (Adj + I) D^-1/2 (built on host from
    the edge list; graphs are equal-sized and edges never cross graphs).
  * p-encoder only feeds its *mean-pooled* graph embedding forward, so with
    c = A_hat^T 1 / NP (> 0) and relu(c*x) = c*relu(x):
        p_graph = (sum_i relu([diag(c) A_hat @ [p_x|1]] @ W01big)_i) @ pW2 + pb2
    where W01big = [[pW0@pW1], [pb0@pW1], [pb1]].  One 17-wide adjacency
    matmul replaces both 128-wide GCN layers + pooling.
  * v-encoder needs per-node embeddings; same trick folds layer-0 into the
    adjacency matmul; layer-2 is a dense adjacency matmul on h1.
  * The head concat is split into per-source matmuls; graph-level terms are
    broadcast back to nodes with a 0/1 graph-expansion matmul.

All matmuls run on TensorE: the p-adjacency stream is fp8-e4m3 (x256 scale
folded into the following weights), v/head streams are bf16, accumulation is
f32 in PSUM.  Biases / relu / leaky-relu / graph-sum pooling are fused into
ScalarE activation drains; consts load as two packed blob DMAs.  v-nodes are
padded 50 -> 64 per graph so every pair of graphs is one aligned 128-row
tile.  Measured ~101us exec across 8 cores at 6.1e-3 rel err.
"""

import os
import numpy as np
from ml_dtypes import bfloat16

B, NP, NV, E = 256, 500, 50, 128
NC = 8
GPC = B // NC          # 32 graphs per core
NVP = 64               # padded v nodes per graph
VN = GPC * NVP         # 2048 padded v nodes per core
WAVES = 8              # p-phase waves per core
GPW = GPC // WAVES     # 8 graphs per wave
PCHUNK = 4             # 512 / 128 p-node chunks per graph


def _mk_specs():
    bf = {}
    off = 0
    for name, P, F in [("vxt", 128, 16 * 17), ("w01v", 18, 128),
                       ("w01", 18, 128), ("avt", 128, 16 * 128), ("vones", 1, VN)]:
        bf[name] = (P, F, off)
        off += F
    bcols = off
    fs = {}
    off = 0
    for name, P, F in [("vxTa", 17, VN), ("gexp", GPC, VN),
                       ("w0bv", 17, 128), ("vw2", 128, 128), ("vb2", 128, 1),
                       ("pw2", 128, 128), ("pb2", 128, 1),
                       ("ha1", 128, 256), ("hbm", 128, 256),
                       ("hc1", 128, 256), ("hd1", 128, 256),
                       ("hb1c", 128, 2), ("hw2", 128, 256),
                       ("hb2c", 128, 1), ("hw3", 128, 1), ("hb3c", 1, 1)]:
        fs[name] = (P, F, off)
        off += F
    return bf, bcols, fs, off


BF16_SPEC, BF16_BLOB_COLS, F32_SPEC, F32_BLOB_COLS = _mk_specs()

# module-level stash for profiling info (read by test.py)
LAST_RESULTS = None

_nc_cache = None


def _build_nc():
    import concourse.bass as bass
    import concourse.bacc as bacc
    import concourse.mybir as mybir
    from concourse.tile import TileContext

    dt = mybir.dt
    f32, bf16, f32r = dt.float32, dt.bfloat16, dt.float32r
    AF = mybir.ActivationFunctionType
    AX = mybir.AxisListType
    OP = mybir.AluOpType

    nc = bacc.Bacc("TRN2", target_bir_lowering=False, debug=False)

    def inp(name, shape, dtype):
        return nc.declare_dram_parameter(name, list(shape), dtype, isOutput=False)

    pA = inp("pA", (WAVES, 128, GPW * PCHUNK * 500), dt.float8e4)   # Ac^T tiles (x256)
    pxt = inp("pxt", (WAVES, 128, GPW * PCHUNK * 17), dt.float8e4)  # p_x aug lhsT tiles
    crall = inp("crall", (WAVES, 1, GPW * 500), bf16)        # c' rows per wave
    bblob = inp("bblob", (128, BF16_BLOB_COLS), bf16)        # packed bf16 consts
    vones = inp("vones", (1, VN), bf16)
    fblob = inp("fblob", (128, F32_BLOB_COLS), f32)          # packed f32 consts
    out_p = nc.declare_dram_parameter("out", [1, VN], f32, isOutput=True)

    with TileContext(nc) as tc:
        with (
            tc.tile_pool(name="const", bufs=1) as cp,
            tc.tile_pool(name="pa", bufs=3) as pap,
            tc.tile_pool(name="wavep", bufs=3) as wp,
            tc.tile_pool(name="scr", bufs=4) as scrp,
            tc.tile_pool(name="big", bufs=1) as bp,
            tc.tile_pool(name="psA", bufs=2, space="PSUM") as psA,
            tc.tile_pool(name="psB", bufs=3, space="PSUM") as psB,
            tc.tile_pool(name="psC", bufs=3, space="PSUM") as psC,
        ):
            # bf16 blob: one DMA, slice views
            bb = cp.tile([128, BF16_BLOB_COLS], bf16, tag="bblob", name="bb")
            bchunks = [528, 1040, 1552, 2064, BF16_BLOB_COLS]
            prev = 0
            for c1 in bchunks:
                nc.sync.dma_start(out=bb[:, prev:c1], in_=bblob[:, prev:c1])
                prev = c1
            yav_t = bp.tile([18, VN], bf16, tag="yav")
            nc.sync.dma_start(out=yav_t[17:18, :], in_=vones[:])
            fb = cp.tile([128, F32_BLOB_COLS], f32, tag="fblob", name="fb")

            def bslc(name):
                P, F, off = BF16_SPEC[name]
                return bb[0:P, off:off + F]

            def fslc(name):
                P, F, off = F32_SPEC[name]
                return fb[0:P, off:off + F]

            def frnd(name, eng="dve", dtype=None):
                P, F, off = F32_SPEC[name]
                dtype = dtype or f32r
                t = cp.tile([P, F], dtype, tag=f"r_{name}", name=f"r_{name}")
                if eng == "act":
                    nc.scalar.activation(out=t[:], in_=fb[0:P, off:off + F],
                                         func=AF.Copy)
                else:
                    nc.vector.tensor_copy(out=t[:], in_=fb[0:P, off:off + F])
                return t

            avt_t = bslc("avt")
            vxt_t = bslc("vxt")
            w01v_t = bslc("w01v")
            w01_t = bslc("w01")
            vb2_t = fslc("vb2")
            pw2_t = fslc("pw2")
            pb2_t = fslc("pb2")
            hc1_t = fslc("hc1")
            hd1_t = fslc("hd1")
            hb1c_t = fslc("hb1c")
            hb2c_t = fslc("hb2c")
            hb3c_t = fslc("hb3c")
            Z = bp.tile([128, GPC], f32, tag="Z")

            # ---------------- p encoder ----------------
            def pwave(wv):
                pa_t = pap.tile([128, GPW * PCHUNK * 500], dt.float8e4, tag="pa")
                nc.sync.dma_start(out=pa_t[:], in_=pA[wv])
                px_t = wp.tile([128, GPW * PCHUNK * 17], dt.float8e4, tag="px")
                nc.sync.dma_start(out=px_t[:], in_=pxt[wv])
                ya_t = wp.tile([18, GPW * 500], bf16, tag="ya")
                nc.sync.dma_start(out=ya_t[17:18, :], in_=crall[wv])
                for gj in range(GPW):
                    yps = psA.tile([17, 500], f32, tag="mA", name="yps")
                    for k in range(PCHUNK):
                        j = gj * PCHUNK + k
                        nc.tensor.matmul(
                            out=yps[:],
                            lhsT=px_t[:, j * 17:(j + 1) * 17],
                            rhs=pa_t[:, j * 500:(j + 1) * 500],
                            start=(k == 0),
                            stop=(k == PCHUNK - 1),
                        )
                    nc.vector.tensor_copy(
                        out=ya_t[0:17, gj * 500:(gj + 1) * 500], in_=yps[:]
                    )
                for gj in range(GPW):
                    qps = psB.tile([128, 500], f32, tag="mB", name="qps")
                    g = wv * GPW + gj
                    nc.tensor.matmul(
                        out=qps[:],
                        lhsT=w01_t[:],
                        rhs=ya_t[:, gj * 500:(gj + 1) * 500],
                        start=True,
                        stop=True,
                    )
                    scr = scrp.tile([128, 500], f32, tag="scr", name="scr")
                    nc.scalar.activation(
                        out=scr[:],
                        in_=qps[:],
                        func=AF.Relu,
                        accum_out=Z[:, g:g + 1],
                    )

            # ---------------- v encoder ----------------
            for pr in range(16):
                yvps = psA.tile([17, 128], f32, tag="mA", name="yvps")
                nc.tensor.matmul(
                    out=yvps[:],
                    lhsT=vxt_t[:, pr * 17:(pr + 1) * 17],
                    rhs=avt_t[:, pr * 128:(pr + 1) * 128],
                    start=True,
                    stop=True,
                )
                nc.vector.tensor_copy(
                    out=yav_t[0:17, pr * 128:(pr + 1) * 128], in_=yvps[:]
                )
            for c0 in range(0, F32_BLOB_COLS, 2048):
                w = min(2048, F32_BLOB_COLS - c0)
                nc.sync.dma_start(out=fb[:, c0:c0 + w], in_=fblob[:, c0:c0 + w])
            w0bv_t = frnd("w0bv", dtype=bf16)
            vw2_t = frnd("vw2", dtype=bf16)
            vxTa_t = frnd("vxTa", "act", dtype=bf16)
            ha1_t = frnd("ha1", "act", dtype=bf16)
            hbm_t = frnd("hbm", "act", dtype=bf16)
            hw2_t = frnd("hw2", "act", dtype=bf16)
            hw3_t = frnd("hw3", dtype=bf16)
            gexp_t = frnd("gexp", "act", dtype=bf16)
            h1_t = bp.tile([128, VN], bf16, tag="h1")
            for ch in range(16):
                qvps = psB.tile([128, 128], f32, tag="mB", name="qvps")
                nc.tensor.matmul(
                    out=qvps[:],
                    lhsT=yav_t[:, ch * 128:(ch + 1) * 128],
                    rhs=w01v_t[:],
                    start=True,
                    stop=True,
                )
                nc.scalar.activation(
                    out=h1_t[:, ch * 128:(ch + 1) * 128], in_=qvps[:], func=AF.Relu
                )
            av1_t = bp.tile([128, VN], bf16, tag="av1")
            for pr in range(16):
                aps = psB.tile([128, 128], f32, tag="mB", name="aps")
                nc.tensor.matmul(
                    out=aps[:],
                    lhsT=h1_t[:, pr * 128:(pr + 1) * 128],
                    rhs=avt_t[:, pr * 128:(pr + 1) * 128],
                    start=True,
                    stop=True,
                )
                nc.vector.tensor_copy(out=av1_t[:, pr * 128:(pr + 1) * 128], in_=aps[:])
            h2_t = bp.tile([128, VN], bf16, tag="h2")
            h0_t = bp.tile([128, VN], bf16, tag="h0")
            for nb in range(4):
                s = slice(nb * 512, (nb + 1) * 512)
                hps = psC.tile([128, 512], f32, tag="mC", name="hps")
                nc.tensor.matmul(
                    out=hps[:], lhsT=vw2_t[:], rhs=av1_t[:, s], start=True, stop=True
                )
                nc.scalar.activation(
                    out=h2_t[:, s], in_=hps[:], func=AF.Identity, bias=vb2_t[:, 0:1]
                )
                h0ps = psC.tile([128, 512], f32, tag="mC", name="h0ps")
                nc.tensor.matmul(
                    out=h0ps[:], lhsT=w0bv_t[:], rhs=vxTa_t[:, s], start=True, stop=True
                )
                nc.vector.tensor_copy(out=h0_t[:, s], in_=h0ps[:])
            Sv = bp.tile([128, GPC], f32, tag="Sv")
            nc.vector.tensor_reduce(
                out=Sv[:],
                in_=h2_t[:].rearrange("p (g n) -> p g n", n=NVP)[:, :, 0:NV],
                axis=AX.X,
                op=OP.add,
            )

            for _wv in range(WAVES):
                pwave(_wv)

            # ---------------- p graph head + graph terms ----------------
            pgps = psA.tile([128, GPC], f32, tag="mA", name="pgps")
            nc.tensor.matmul(out=pgps[:], lhsT=pw2_t[:], rhs=Z[:], start=True, stop=True)
            pg_t = bp.tile([128, GPC], f32, tag="pg")
            nc.scalar.activation(
                out=pg_t[:], in_=pgps[:], func=AF.Identity, bias=pb2_t[:, 0:1]
            )
            ggps = psA.tile([32, 256], f32, tag="mA", name="ggps")
            nc.tensor.matmul(
                out=ggps[:], lhsT=Sv[:], rhs=hc1_t[:], start=True, stop=False
            )
            nc.tensor.matmul(
                out=ggps[:], lhsT=pg_t[:], rhs=hd1_t[:], start=False, stop=True
            )
            gg_t = bp.tile([32, 256], bf16, tag="gg")
            nc.vector.tensor_copy(out=gg_t[:], in_=ggps[:])

            # ---------------- head ----------------
            xh_ts = [bp.tile([128, VN], bf16, tag=f"xh{b}", name=f"xh{b}")
                     for b in range(2)]
            for blk in range(2):
                bs = slice(blk * 128, (blk + 1) * 128)
                for nb in range(4):
                    s = slice(nb * 512, (nb + 1) * 512)
                    xps = psC.tile([128, 512], f32, tag="mC", name="xps")
                    nc.tensor.matmul(
                        out=xps[:], lhsT=ha1_t[:, bs], rhs=h2_t[:, s],
                        start=True, stop=False,
                    )
                    nc.tensor.matmul(
                        out=xps[:], lhsT=hbm_t[:, bs], rhs=h0_t[:, s],
                        start=False, stop=False,
                    )
                    nc.tensor.matmul(
                        out=xps[:], lhsT=gg_t[:, bs], rhs=gexp_t[:, s],
                        start=False, stop=True,
                    )
                    nc.scalar.activation(
                        out=xh_ts[blk][:, s], in_=xps[:], func=AF.Lrelu,
                        bias=hb1c_t[:, blk:blk + 1], alpha=0.01,
                    )
            hm_t = bp.tile([128, VN], bf16, tag="hm")
            for nb in range(4):
                s = slice(nb * 512, (nb + 1) * 512)
                hps2 = psC.tile([128, 512], f32, tag="mC", name="hps2")
                nc.tensor.matmul(
                    out=hps2[:], lhsT=hw2_t[:, 0:128], rhs=xh_ts[0][:, s],
                    start=True, stop=False,
                )
                nc.tensor.matmul(
                    out=hps2[:], lhsT=hw2_t[:, 128:256], rhs=xh_ts[1][:, s],
                    start=False, stop=True,
                )
                nc.scalar.activation(
                    out=hm_t[:, s], in_=hps2[:], func=AF.Lrelu,
                    bias=hb2c_t[:, 0:1], alpha=0.01,
                )
            ob = bp.tile([1, VN], f32, tag="ob")
            for nb in range(4):
                s = slice(nb * 512, (nb + 1) * 512)
                lps = psA.tile([1, 512], f32, tag="mA", name="lps")
                nc.tensor.matmul(
                    out=lps[:], lhsT=hw3_t[:], rhs=hm_t[:, s], start=True, stop=True
                )
                nc.scalar.activation(
                    out=ob[:, s], in_=lps[:], func=AF.Identity, bias=hb3c_t[:, 0:1]
                )
            nc.sync.dma_start(out=out_p[:], in_=ob[:])

    nc.compile()
    return nc


def _host_prep(inp):
    f32 = np.float32
    px = np.asarray(inp["p_x"], f32)
    vx = np.asarray(inp["v_x"], f32)
    pei = np.asarray(inp["p_edge_index"]).astype(np.int64)
    vei = np.asarray(inp["v_edge_index"]).astype(np.int64)
    g = {k: np.asarray(inp[k], f32) for k in
         ("pW0", "pb0", "pW1", "pb1", "pW2", "pb2",
          "vW0", "vb0", "vW1", "vb1", "vW2", "vb2",
          "hW1", "hb1", "hW2", "hb2", "hW3", "hb3")}

    # ---- p-side adjacency (with pooling weights folded) ----
    psrc, pdst = pei[0], pei[1]
    pdeg = 1.0 + np.bincount(pdst, minlength=B * NP).astype(f32)
    pdinv = (1.0 / np.sqrt(pdeg)).astype(f32)
    # c = A_hat^T 1  (column sums incl. self loop), then / NP
    csum = pdinv * np.bincount(psrc, weights=pdinv[pdst], minlength=B * NP).astype(f32)
    cp = (csum + pdinv * pdinv) / NP                                  # [B*NP]
    AcT = np.zeros((B, 512, 500), f32)
    w = (pdinv[psrc] * pdinv[pdst] * cp[pdst]).astype(f32)
    np.add.at(AcT, (pdst // NP, psrc % NP, pdst % NP), w)
    ar = np.arange(B * NP)
    AcT[ar // NP, ar % NP, ar % NP] += pdinv * pdinv * cp
    # [core, wave, gj, chunk, p, d] -> [core, wave, p, gj, chunk, d]
    from ml_dtypes import float8_e4m3
    pa = (np.ascontiguousarray(
        AcT.reshape(NC, WAVES, GPW, PCHUNK, 128, 500).transpose(0, 1, 4, 2, 3, 5)
    ).reshape(NC, WAVES, 128, GPW * PCHUNK * 500) * 256.0).astype(float8_e4m3)

    pxa = np.zeros((B, 512, 17), f32)
    pxa[:, :NP, :16] = px.reshape(B, NP, 16)
    pxa[:, :NP, 16] = 1.0
    pxt = np.ascontiguousarray(
        pxa.reshape(NC, WAVES, GPW, PCHUNK, 128, 17).transpose(0, 1, 4, 2, 3, 5)
    ).reshape(NC, WAVES, 128, GPW * PCHUNK * 17).astype(float8_e4m3)

    crall = np.ascontiguousarray(cp.reshape(NC, WAVES, 1, GPW * 500)).astype(bfloat16)

    # ---- v-side adjacency (padded to 64/graph, pairs of graphs) ----
    vsrc, vdst = vei[0], vei[1]
    vdeg = 1.0 + np.bincount(vdst, minlength=B * NV).astype(f32)
    vdinv = (1.0 / np.sqrt(vdeg)).astype(f32)
    AvT = np.zeros((B, NVP, NVP), f32)
    wv = (vdinv[vsrc] * vdinv[vdst]).astype(f32)
    np.add.at(AvT, (vdst // NV, vsrc % NV, vdst % NV), wv)
    arv = np.arange(B * NV)
    AvT[arv // NV, arv % NV, arv % NV] += vdinv * vdinv
    avt_pair = np.zeros((B // 2, 128, 128), f32)
    avt_pair[:, :NVP, :NVP] = AvT[0::2]
    avt_pair[:, NVP:, NVP:] = AvT[1::2]
    # [core, pair, p, d] -> [core, p, pair*128+d]
    avt = np.ascontiguousarray(
        avt_pair.reshape(NC, 16, 128, 128).transpose(0, 2, 1, 3)
    ).reshape(NC, 128, 16 * 128).astype(bfloat16)

    vxa = np.zeros((B, NVP, 17), f32)
    vxa[:, :NV, :16] = vx.reshape(B, NV, 16)
    vxa[:, :NV, 16] = 1.0
    vxt = np.ascontiguousarray(
        vxa.reshape(NC, 16, 128, 17).transpose(0, 2, 1, 3)
    ).reshape(NC, 128, 16 * 17).astype(bfloat16)
    vxTa = np.ascontiguousarray(
        vxa.reshape(NC, VN, 17).transpose(0, 2, 1)
    ).astype(f32)

    gexp = np.zeros((GPC, VN), f32)
    for gi in range(GPC):
        gexp[gi, gi * NVP:(gi + 1) * NVP] = 1.0

    # ---- weights ----
    w01 = np.concatenate(
        [(g["pW0"] @ g["pW1"]) / 256.0, (g["pb0"] @ g["pW1"])[None] / 256.0,
         g["pb1"][None]], 0
    ).astype(bfloat16)
    w01v = np.concatenate(
        [g["vW0"] @ g["vW1"], (g["vb0"] @ g["vW1"])[None], g["vb1"][None]], 0
    ).astype(bfloat16)
    w0bv = np.concatenate([g["vW0"], g["vb0"][None]], 0).astype(f32)
    hW1, hW2 = g["hW1"], g["hW2"]
    hw2c = np.ascontiguousarray(
        hW2.reshape(2, 128, 128).transpose(1, 0, 2)
    ).reshape(128, 256).astype(f32)

    fconsts = {
        "gexp": gexp,
        "w0bv": w0bv,
        "vw2": g["vW2"].astype(f32),
        "vb2": g["vb2"].reshape(128, 1).astype(f32),
        "pw2": g["pW2"].astype(f32),
        "pb2": g["pb2"].reshape(128, 1).astype(f32),
        "ha1": hW1[0:128].astype(f32),
        "hbm": hW1[128:256].astype(f32),
        "hc1": (hW1[256:384] / NV).astype(f32),
        "hd1": hW1[384:512].astype(f32),
        "hb1c": np.ascontiguousarray(g["hb1"].reshape(2, 128).T).astype(f32),
        "hw2": hw2c,
        "hb2c": g["hb2"].reshape(128, 1).astype(f32),
        "hw3": g["hW3"].astype(f32),
        "hb3c": g["hb3"].reshape(1, 1).astype(f32),
    }
    bconsts = {
        "w01v": w01v,
        "w01": w01,
        "vones": np.ones((1, VN), bfloat16),
    }
    in_maps = []
    for c in range(NC):
        bblob = np.zeros((128, BF16_BLOB_COLS), bfloat16)
        for name, arr in {**bconsts, "avt": avt[c], "vxt": vxt[c]}.items():
            P, F, off = BF16_SPEC[name]
            bblob[0:P, off:off + F] = arr
        fblob = np.zeros((128, F32_BLOB_COLS), f32)
        for name, arr in {**fconsts, "vxTa": vxTa[c]}.items():
            P, F, off = F32_SPEC[name]
            fblob[0:P, off:off + F] = arr
        m = {
            "pA": pa[c],
            "pxt": pxt[c],
            "crall": crall[c],
            "bblob": bblob,
            "fblob": fblob,
            "vones": bconsts["vones"],
        }
        in_maps.append(m)
    return in_maps


def _ensure_ntff_hook():
    """Provide antenv.axon_hooks if the image lacks it, so trace=True works."""
    try:
        from antenv.axon_hooks import get_axon_ntff_profile_hook  # noqa: F401
        return
    except ImportError:
        pass
    try:
        import sys
        import types
        import antenv
        from trn_agent_boot.trn_boot import _ntff_profile_via_ctypes

        hook = _ntff_profile_via_ctypes("/opt/axon/libaxon_pjrt.so")
        mod = types.ModuleType("antenv.axon_hooks")
        mod._hook = hook
        mod.get_axon_ntff_profile_hook = lambda: mod._hook
        mod.set_axon_ntff_profile_hook = lambda h: setattr(mod, "_hook", h)
        sys.modules["antenv.axon_hooks"] = mod
        antenv.axon_hooks = mod
    except Exception:
        pass


def kernel(**inputs):
    global _nc_cache, LAST_RESULTS
    from concourse.bass_utils import run_bass_kernel_spmd

    in_maps = _host_prep(inputs)
    if _nc_cache is None:
        _nc_cache = _build_nc()
    trace = os.environ.get("KERNEL_TRACE", "0") == "1"
    if trace:
        _ensure_ntff_hook()
    res = run_bass_kernel_spmd(_nc_cache, in_maps, core_ids=list(range(NC)),
                               trace=trace)
    LAST_RESULTS = res
    outs = [res.results[c]["out"].reshape(GPC, NVP)[:, :NV] for c in range(NC)]
    return np.concatenate(outs, 0).astype(np.float32)



# revision 11
# speedup vs baseline: 1.2239x; 1.2239x over previous
"""Trainium2 Bass kernel for nn_ActorNetwork (GNN message passing), 8 NeuronCores.

Strategy (v2)
-------------
Data-parallel over the 256 graphs: core c owns graphs [32c, 32c+32).

Algebraic restructure (validated vs reference to ~5.2e-3 rel err):
  * GCNConv aggregation as dense per-graph matmul with the block-diagonal
    normalized adjacency (built on host); p-encoder collapses both GCN
    layers + mean-pool into ONE fp8 adjacency matmul (c'-scaling folded
    into A, bias row via a fake source node carrying c') followed by an
    18->128 projection, relu and a per-graph column-sum (Z).
  * fp8 DoubleRow perf mode on the adjacency matmuls (K=256 per pass)
    halves TensorE streaming time for the p phase.
  * Head fully refactored: h0/h2 are never materialized.  All graph-level
    terms fold into gg = Sv'^T (vW2 hc1/NV) + Z^T (pW2 hd1); per-node terms
    use av1 (K=128) plus one combined K=66 matmul whose stationary stacks
    [gg | w0bv@hbm | hb1'] against [gexp | vxTa | ones].  Constant offsets
    (vb2/pb2 paths) fold into hb1' on the host.
  * PSUM drains batched to 512-1024 wide tiles; relu+accum split across
    ScalarE (activation w/ accum) and DVE (tensor_scalar max w/ accum);
    head processed in two 16-graph halves so it overlaps p-waves 4-7.
"""

import os
import numpy as np
from ml_dtypes import bfloat16, float8_e4m3

B, NP, NV, E = 256, 500, 50, 128
NC = 8
GPC = B // NC          # 32 graphs per core
NVP = 64               # padded v nodes per graph
VN = GPC * NVP         # 2048 padded v nodes per core
WAVES = 8              # p-phase waves per core
GPW = GPC // WAVES     # 4 graphs per wave
PCHUNK = 4             # 512/128 p-node chunks per graph

# bf16 const blob column layout
_BSPEC = {}
_off = 0
for _name, _p, _f in [("vxt", 128, 16 * 17), ("avt", 128, 16 * 128),
                      ("w01v", 18, 128), ("w01", 18, 128),
                      ("A1", 128, 256), ("C1", 128, 256), ("D1", 128, 256),
                      ("hw2", 128, 256), ("hw3", 128, 1)]:
    _BSPEC[_name] = (_p, _f, _off)
    _off += _f
BCOLS = _off
BSPLIT = _BSPEC["A1"][2]   # DMA chunk boundary: v-consts | head-consts

LAST_RESULTS = None
_nc_cache = None


def _build_nc():
    import concourse.bass as bass  # noqa: F401
    import concourse.bacc as bacc
    import concourse.mybir as mybir
    from concourse.tile import TileContext

    dt = mybir.dt
    f32, bf16 = dt.float32, dt.bfloat16
    AF = mybir.ActivationFunctionType
    AX = mybir.AxisListType
    OP = mybir.AluOpType
    DR = mybir.MatmulPerfMode.DoubleRow

    nc = bacc.Bacc("TRN2", target_bir_lowering=False, debug=False)

    def inp(name, shape, dtype):
        return nc.declare_dram_parameter(name, list(shape), dtype, isOutput=False)

    pA = inp("pA", (WAVES, 128, GPW * PCHUNK * 500), dt.float8e4)
    pxp = inp("pxp", (128, GPC * 128), dt.float8e4)
    bblob = inp("bblob", (128, BCOLS), bf16)
    vxg_d = inp("vxg", (66, VN), bf16)
    vvgc = inp("vvgc", (50, 256), bf16)
    vones = inp("vones", (1, VN), bf16)
    sblob = inp("sblob", (128, 2), f32)
    out_p = nc.declare_dram_parameter("out", [1, VN], f32, isOutput=True)

    with TileContext(nc) as tc:
        with (
            tc.tile_pool(name="const", bufs=1) as cp,
            tc.tile_pool(name="pa", bufs=3) as pap,
            tc.tile_pool(name="ya", bufs=3) as yap,
            tc.tile_pool(name="scrS", bufs=2) as scrS,
            tc.tile_pool(name="scrV", bufs=2) as scrV,
            tc.tile_pool(name="big", bufs=1) as bp,
            tc.tile_pool(name="psY", bufs=2, space="PSUM") as psY,
            tc.tile_pool(name="psQ", bufs=2, space="PSUM") as psQ,
            tc.tile_pool(name="psV", bufs=2, space="PSUM") as psV,
        ):
            bb = cp.tile([128, BCOLS], bf16, tag="bblob", name="bb")
            nc.sync.dma_start(out=bb[:, 0:BSPLIT], in_=bblob[:, 0:BSPLIT])
            pxp_t = cp.tile([128, GPC * 128], dt.float8e4, tag="pxp", name="pxp")
            nc.sync.dma_start(out=pxp_t[:], in_=pxp[:])
            vxg = cp.tile([66, VN], bf16, tag="vxg", name="vxg")
            nc.sync.dma_start(out=vxg[:], in_=vxg_d[:])
            vvg = cp.tile([66, 256], bf16, tag="vvg", name="vvg")
            nc.sync.dma_start(out=vvg[16:66, :], in_=vvgc[:])
            sb = cp.tile([128, 2], f32, tag="sblob", name="sb")
            nc.sync.dma_start(out=sb[:], in_=sblob[:])
            nc.sync.dma_start(out=bb[:, BSPLIT:BCOLS], in_=bblob[:, BSPLIT:BCOLS])

            def bslc(name):
                P, F, off = _BSPEC[name]
                return bb[0:P, off:off + F]

            vxt_t, avt_t = bslc("vxt"), bslc("avt")
            w01v_t, w01_t = bslc("w01v"), bslc("w01")
            A1_t, C1_t, D1_t = bslc("A1"), bslc("C1"), bslc("D1")
            hw2_t, hw3_t = bslc("hw2"), bslc("hw3")

            Z = bp.tile([128, GPC], f32, tag="Z")
            Sv = bp.tile([128, GPC], f32, tag="Sv")
            Zb = bp.tile([128, GPC], bf16, tag="Zb")
            Svb = bp.tile([128, GPC], bf16, tag="Svb")
            yav = bp.tile([18, VN], bf16, tag="yav")
            nc.sync.dma_start(out=yav[17:18, :], in_=vones[:])
            h1 = bp.tile([128, VN], bf16, tag="h1")
            av1 = bp.tile([128, VN], bf16, tag="av1")
            xh0 = bp.tile([128, VN], bf16, tag="xh0")
            xh1 = bp.tile([128, VN], bf16, tag="xh1")
            hm = bp.tile([128, VN], bf16, tag="hm")
            ob = bp.tile([1, VN], f32, tag="ob")

            # ---------------- v encoder ----------------
            def v_encoder():
                for qb in range(4):
                    yvt = psV.tile([17, 512], f32, tag="psv", name=f"yv{qb}")
                    for j in range(4):
                        pb = qb * 4 + j
                        nc.tensor.matmul(
                            out=yvt[:, j * 128:(j + 1) * 128],
                            lhsT=vxt_t[:, pb * 17:(pb + 1) * 17],
                            rhs=avt_t[:, pb * 128:(pb + 1) * 128],
                            start=True, stop=True,
                        )
                    nc.vector.tensor_copy(
                        out=yav[0:17, qb * 512:(qb + 1) * 512], in_=yvt[:])
                for qb in range(4):
                    qvt = psV.tile([128, 512], f32, tag="psv", name=f"qv{qb}")
                    for j in range(4):
                        ch = qb * 4 + j
                        nc.tensor.matmul(
                            out=qvt[:, j * 128:(j + 1) * 128],
                            lhsT=yav[:, ch * 128:(ch + 1) * 128],
                            rhs=w01v_t[:],
                            start=True, stop=True,
                        )
                    nc.scalar.activation(
                        out=h1[:, qb * 512:(qb + 1) * 512], in_=qvt[:], func=AF.Relu)
                for qb in range(4):
                    apt = psV.tile([128, 512], f32, tag="psv", name=f"ap{qb}")
                    for j in range(4):
                        pb = qb * 4 + j
                        nc.tensor.matmul(
                            out=apt[:, j * 128:(j + 1) * 128],
                            lhsT=h1[:, pb * 128:(pb + 1) * 128],
                            rhs=avt_t[:, pb * 128:(pb + 1) * 128],
                            start=True, stop=True,
                        )
                    nc.vector.tensor_copy(
                        out=av1[:, qb * 512:(qb + 1) * 512], in_=apt[:])
                nc.vector.tensor_reduce(
                    out=Sv[:],
                    in_=av1[:].rearrange("p (g n) -> p g n", n=NVP),
                    axis=AX.X, op=OP.add,
                )

            # ---------------- p waves ----------------
            def p_mm1_batch(wv, i):
                """DR adjacency matmuls for graphs (wv*4 + 2i, +1) -> yt."""
                pa_t = _pa_tiles[wv]
                yt = psY.tile([18, 1024], f32, tag="yt", name=f"yt{wv}_{i}")
                for half in range(2):
                    gj = i * 2 + half
                    g = wv * GPW + gj
                    for p in range(2):
                        j = gj * PCHUNK + p * 2
                        lhs3 = pxp_t[:, g * 128 + p * 64: g * 128 + (p + 1) * 64]
                        lhs3 = lhs3.rearrange("p (two m) -> p two m", two=2)[:, :, 0:18]
                        rhs3 = pa_t[:, j * 500:(j + 2) * 500]
                        rhs3 = rhs3.rearrange("p (two n) -> p two n", two=2)
                        nc.tensor.matmul(
                            out=yt[:, half * 512: half * 512 + 500],
                            lhsT=lhs3, rhs=rhs3,
                            start=(p == 0), stop=(p == 1),
                            perf_mode=DR,
                        )
                return yt

            def p_drain_batch(wv, i, yt):
                bi = wv * 2 + i
                ya = yap.tile([18, 1024], bf16, tag="ya", name=f"ya{bi}")
                if bi % 2 == 0:
                    nc.scalar.activation(out=ya[:], in_=yt[:], func=AF.Copy)
                else:
                    nc.vector.tensor_copy(out=ya[:], in_=yt[:])
                for half in range(2):
                    g = wv * GPW + i * 2 + half
                    qt = psQ.tile([128, 500], f32, tag="qt", name=f"qt{g}")
                    nc.tensor.matmul(
                        out=qt[:], lhsT=w01_t[:],
                        rhs=ya[:, half * 512: half * 512 + 500],
                        start=True, stop=True,
                    )
                    if g % 8 < 3:     # 12 of 32 on ScalarE
                        scr = scrS.tile([128, 500], bf16, tag="scrS", name=f"sS{g}")
                        nc.scalar.activation(
                            out=scr[:], in_=qt[:], func=AF.Relu,
                            accum_out=Z[:, g:g + 1])
                    else:             # 20 of 32 on DVE
                        scr = scrV.tile([128, 500], bf16, tag="scrV", name=f"sV{g}")
                        nc.vector.tensor_scalar(
                            out=scr[:], in0=qt[:], scalar1=0.0, scalar2=None,
                            op0=OP.max, op1=OP.add, accum_out=Z[:, g:g + 1])

            # ---------------- head (two 16-graph halves) ----------------
            def head_half(h):
                base = h * 32
                ggt = psV.tile([48, 256], f32, tag="psv", name=f"gg{h}")
                gsl = slice(base, base + 16)
                hsl = slice(h * 16, (h + 1) * 16)
                nc.vector.tensor_copy(out=Svb[:, hsl], in_=Sv[:, hsl])
                nc.vector.tensor_copy(out=Zb[:, hsl], in_=Z[:, hsl])
                nc.tensor.matmul(
                    out=ggt[gsl, :], lhsT=Svb[:, hsl],
                    rhs=C1_t[:], start=True, stop=False)
                nc.tensor.matmul(
                    out=ggt[gsl, :], lhsT=Zb[:, hsl],
                    rhs=D1_t[:], start=False, stop=True)
                nc.vector.tensor_copy(out=vvg[gsl, :], in_=ggt[gsl, :])
                for blk in range(2):
                    xh = (xh0, xh1)[blk]
                    bs = slice(blk * 128, (blk + 1) * 128)
                    for nb in range(2):
                        s = slice(h * 1024 + nb * 512, h * 1024 + (nb + 1) * 512)
                        xt = psV.tile([128, 512], f32, tag="psv",
                                      name=f"xt{h}{blk}{nb}")
                        nc.tensor.matmul(out=xt[:], lhsT=A1_t[:, bs],
                                         rhs=av1[:, s], start=True, stop=False)
                        nc.tensor.matmul(out=xt[:], lhsT=vvg[:, bs],
                                         rhs=vxg[:, s], start=False, stop=True)
                        nc.scalar.activation(
                            out=xh[:, s], in_=xt[:], func=AF.Lrelu, alpha=0.01)
                for nb in range(2):
                    s = slice(h * 1024 + nb * 512, h * 1024 + (nb + 1) * 512)
                    ht = psV.tile([128, 512], f32, tag="psv", name=f"ht{h}{nb}")
                    nc.tensor.matmul(out=ht[:], lhsT=hw2_t[:, 0:128],
                                     rhs=xh0[:, s], start=True, stop=False)
                    nc.tensor.matmul(out=ht[:], lhsT=hw2_t[:, 128:256],
                                     rhs=xh1[:, s], start=False, stop=True)
                    nc.scalar.activation(
                        out=hm[:, s], in_=ht[:], func=AF.Lrelu,
                        bias=sb[:, 0:1], alpha=0.01)
                for nb in range(2):
                    s = slice(h * 1024 + nb * 512, h * 1024 + (nb + 1) * 512)
                    lt = psV.tile([1, 512], f32, tag="psv", name=f"lt{h}{nb}")
                    nc.tensor.matmul(out=lt[:], lhsT=hw3_t[:], rhs=hm[:, s],
                                     start=True, stop=True)
                    nc.scalar.activation(
                        out=ob[:, s], in_=lt[:], func=AF.Identity,
                        bias=sb[0:1, 1:2])
                osl = slice(h * 1024, (h + 1) * 1024)
                nc.sync.dma_start(out=out_p[:, osl], in_=ob[:, osl])

            # ---------------- schedule ----------------
            _pa_tiles = {}

            def start_wave(wv):
                pa_t = pap.tile([128, GPW * PCHUNK * 500], dt.float8e4,
                                tag="pa", name=f"pa{wv}")
                nc.sync.dma_start(out=pa_t[:], in_=pA[wv])
                _pa_tiles[wv] = pa_t

            start_wave(0)
            start_wave(1)
            v_encoder()
            prev = None
            for wv in range(WAVES):
                if wv + 2 < WAVES:
                    start_wave(wv + 2)
                for i in range(2):
                    yt = p_mm1_batch(wv, i)
                    if prev is not None:
                        p_drain_batch(*prev)
                    prev = (wv, i, yt)
                if wv == 4:
                    head_half(0)
            p_drain_batch(*prev)
            head_half(1)

    nc.compile()
    return nc


def _host_prep(inp):
    f32 = np.float32
    px = np.asarray(inp["p_x"], f32)
    vx = np.asarray(inp["v_x"], f32)
    pei = np.asarray(inp["p_edge_index"]).astype(np.int64)
    vei = np.asarray(inp["v_edge_index"]).astype(np.int64)
    g = {k: np.asarray(inp[k], f32) for k in
         ("pW0", "pb0", "pW1", "pb1", "pW2", "pb2",
          "vW0", "vb0", "vW1", "vb1", "vW2", "vb2",
          "hW1", "hb1", "hW2", "hb2", "hW3", "hb3")}

    # ---- p-side adjacency (pool weights + fake bias row folded) ----
    psrc, pdst = pei[0], pei[1]
    pdeg = 1.0 + np.bincount(pdst, minlength=B * NP).astype(f32)
    pdinv = (1.0 / np.sqrt(pdeg)).astype(f32)
    csum = pdinv * np.bincount(psrc, weights=pdinv[pdst], minlength=B * NP).astype(f32)
    cp = (csum + pdinv * pdinv) / NP
    AcT = np.zeros((B, 512, 500), f32)
    w = (pdinv[psrc] * pdinv[pdst] * cp[pdst]).astype(f32)
    np.add.at(AcT, (pdst // NP, psrc % NP, pdst % NP), w)
    ar = np.arange(B * NP)
    AcT[ar // NP, ar % NP, ar % NP] += pdinv * pdinv * cp
    AcT[:, 500, :] = cp.reshape(B, NP)
    pa = (np.ascontiguousarray(
        AcT.reshape(NC, WAVES, GPW, PCHUNK, 128, 500).transpose(0, 1, 4, 2, 3, 5)
    ).reshape(NC, WAVES, 128, GPW * PCHUNK * 500) * 256.0).astype(float8_e4m3)

    pxa = np.zeros((B, 512, 18), f32)
    pxa[:, :NP, :16] = px.reshape(B, NP, 16)
    pxa[:, :NP, 16] = 1.0
    pxa[:, 500, 17] = 1.0
    # [core, 128row, graph, pair, plane, 32col]
    px6 = pxa.reshape(NC, GPC, PCHUNK, 128, 18).transpose(0, 3, 1, 2, 4)
    pxp = np.zeros((NC, 128, GPC, 2, 2, 32), f32)
    pxp[..., 0:18] = px6.reshape(NC, 128, GPC, 2, 2, 18)
    pxp = pxp.reshape(NC, 128, GPC * 128).astype(float8_e4m3)

    # ---- v-side adjacency (padded to 64/graph, pairs of graphs) ----
    vsrc, vdst = vei[0], vei[1]
    vdeg = 1.0 + np.bincount(vdst, minlength=B * NV).astype(f32)
    vdinv = (1.0 / np.sqrt(vdeg)).astype(f32)
    AvT = np.zeros((B, NVP, NVP), f32)
    wv_ = (vdinv[vsrc] * vdinv[vdst]).astype(f32)
    np.add.at(AvT, (vdst // NV, vsrc % NV, vdst % NV), wv_)
    arv = np.arange(B * NV)
    AvT[arv // NV, arv % NV, arv % NV] += vdinv * vdinv
    avt_pair = np.zeros((B // 2, 128, 128), f32)
    avt_pair[:, :NVP, :NVP] = AvT[0::2]
    avt_pair[:, NVP:, NVP:] = AvT[1::2]
    avt = np.ascontiguousarray(
        avt_pair.reshape(NC, 16, 128, 128).transpose(0, 2, 1, 3)
    ).reshape(NC, 128, 16 * 128).astype(bfloat16)

    vxa = np.zeros((B, NVP, 17), f32)
    vxa[:, :NV, :16] = vx.reshape(B, NV, 16)
    vxa[:, :NV, 16] = 1.0
    vxt = np.ascontiguousarray(
        vxa.reshape(NC, 16, 128, 17).transpose(0, 2, 1, 3)
    ).reshape(NC, 128, 16 * 17).astype(bfloat16)
    vxTa = np.ascontiguousarray(
        vxa.reshape(NC, VN, 17).transpose(0, 2, 1)
    ).astype(bfloat16)

    # ---- weights + head folds ----
    w01 = np.concatenate(
        [g["pW0"] @ g["pW1"], (g["pb0"] @ g["pW1"])[None], g["pb1"][None]], 0
    ).astype(f32) / 256.0
    w01v = np.concatenate(
        [g["vW0"] @ g["vW1"], (g["vb0"] @ g["vW1"])[None], g["vb1"][None]], 0)
    hW1, hb1 = g["hW1"], g["hb1"]
    ha1o, hbmo = hW1[0:128], hW1[128:256]
    hc1o, hd1o = hW1[256:384], hW1[384:512]
    w0bv = np.concatenate([g["vW0"], g["vb0"][None]], 0)
    A1 = g["vW2"] @ ha1o
    B1 = w0bv @ hbmo
    C1 = g["vW2"] @ hc1o / NV
    D1 = g["pW2"] @ hd1o
    hb1p = hb1 + g["vb2"] @ (ha1o + hc1o) + g["pb2"] @ hd1o
    hw2c = np.ascontiguousarray(
        g["hW2"].reshape(2, 128, 128).transpose(1, 0, 2)).reshape(128, 256)

    gexp = np.zeros((GPC, VN), f32)
    for gi in range(GPC):
        gexp[gi, gi * NVP:(gi + 1) * NVP] = 1.0

    bconsts = {"w01v": w01v, "w01": w01, "A1": A1, "C1": C1, "D1": D1,
               "hw2": hw2c, "hw3": g["hW3"]}
    # vvg rows 16-65: [16 zero rows][16 zero rows (gg half1 overwrites)][B1][hb1p]
    vvgc = np.concatenate(
        [np.zeros((32, 256), f32), B1, hb1p[None]], 0).astype(bfloat16)  # [50, 256]
    sblob = np.zeros((128, 2), f32)
    sblob[:, 0] = g["hb2"]
    sblob[0, 1] = g["hb3"][0]

    in_maps = []
    for c in range(NC):
        bblob = np.zeros((128, BCOLS), bfloat16)
        for name, arr in {**bconsts, "avt": avt[c], "vxt": vxt[c]}.items():
            P, F, off = _BSPEC[name]
            bblob[0:P, off:off + F] = arr.astype(bfloat16)
        vxg = np.zeros((66, VN), bfloat16)
        vxg[0:16] = gexp[0:16].astype(bfloat16)
        vxg[32:48] = gexp[16:32].astype(bfloat16)
        vxg[48:65] = vxTa[c]
        vxg[65] = 1.0
        in_maps.append({
            "pA": pa[c], "pxp": pxp[c], "bblob": bblob,
            "vxg": vxg, "vvgc": vvgc, "sblob": sblob,
            "vones": np.ones((1, VN), bfloat16),
        })
    return in_maps


def _ensure_ntff_hook():
    """Provide antenv.axon_hooks if the image lacks it, so trace=True works."""
    try:
        from antenv.axon_hooks import get_axon_ntff_profile_hook  # noqa: F401
        return
    except ImportError:
        pass
    try:
        import sys
        import types
        import antenv
        from trn_agent_boot.trn_boot import _ntff_profile_via_ctypes

        hook = _ntff_profile_via_ctypes("/opt/axon/libaxon_pjrt.so")
        mod = types.ModuleType("antenv.axon_hooks")
        mod._hook = hook
        mod.get_axon_ntff_profile_hook = lambda: mod._hook
        mod.set_axon_ntff_profile_hook = lambda h: setattr(mod, "_hook", h)
        sys.modules["antenv.axon_hooks"] = mod
        antenv.axon_hooks = mod
    except Exception:
        pass


def kernel(**inputs):
    global _nc_cache, LAST_RESULTS
    from concourse.bass_utils import run_bass_kernel_spmd

    in_maps = _host_prep(inputs)
    if _nc_cache is None:
        _nc_cache = _build_nc()
    trace = os.environ.get("KERNEL_TRACE", "0") == "1"
    if trace:
        _ensure_ntff_hook()
    res = run_bass_kernel_spmd(_nc_cache, in_maps, core_ids=list(range(NC)),
                               trace=trace)
    LAST_RESULTS = res
    outs = [res.results[c]["out"].reshape(GPC, NVP)[:, :NV] for c in range(NC)]
    return np.concatenate(outs, 0).astype(np.float32)


# revision 16
# speedup vs baseline: 1.4158x; 1.1568x over previous
"""Trainium2 Bass kernel for nn_ActorNetwork (GNN message passing), 8 NeuronCores.

Strategy (v2)
-------------
Data-parallel over the 256 graphs: core c owns graphs [32c, 32c+32).

Algebraic restructure (validated vs reference to ~5.2e-3 rel err):
  * GCNConv aggregation as dense per-graph matmul with the block-diagonal
    normalized adjacency (built on host); p-encoder collapses both GCN
    layers + mean-pool into ONE fp8 adjacency matmul (c'-scaling folded
    into A, bias row via a fake source node carrying c') followed by an
    18->128 projection, relu and a per-graph column-sum (Z).
  * fp8 DoubleRow perf mode on the adjacency matmuls (K=256 per pass)
    halves TensorE streaming time for the p phase.
  * Head fully refactored: h0/h2 are never materialized.  All graph-level
    terms fold into gg = Sv'^T (vW2 hc1/NV) + Z^T (pW2 hd1); per-node terms
    use av1 (K=128) plus one combined K=66 matmul whose stationary stacks
    [gg | w0bv@hbm | hb1'] against [gexp | vxTa | ones].  Constant offsets
    (vb2/pb2 paths) fold into hb1' on the host.
  * PSUM drains batched to 512-1024 wide tiles; relu+accum split across
    ScalarE (activation w/ accum) and DVE (tensor_scalar max w/ accum);
    head processed in two 16-graph halves so it overlaps p-waves 4-7.
"""

import os
import numpy as np
from ml_dtypes import bfloat16, float8_e4m3

B, NP, NV, E = 256, 500, 50, 128
NC = 8
GPC = B // NC          # 32 graphs per core
NVP = 64               # padded v nodes per graph
VN = GPC * NVP         # 2048 padded v nodes per core
WAVES = 8              # p-phase waves per core
GPW = GPC // WAVES     # 4 graphs per wave
PCHUNK = 4             # 512/128 p-node chunks per graph

# bf16 const blob column layout
_BSPEC = {}
_off = 0
for _name, _p, _f in [("vxt", 128, 16 * 17), ("avt", 128, 16 * 128),
                      ("w01v", 18, 128), ("w01", 18, 128),
                      ("A1", 128, 256), ("C1", 128, 256), ("D1", 128, 256),
                      ("hw2", 128, 256), ("hw3", 128, 1)]:
    _BSPEC[_name] = (_p, _f, _off)
    _off += _f
BCOLS = _off
BSPLIT = _BSPEC["A1"][2]   # DMA chunk boundary: v-consts | head-consts

LAST_RESULTS = None
_nc_cache = None


def _build_nc():
    import concourse.bass as bass  # noqa: F401
    import concourse.bacc as bacc
    import concourse.mybir as mybir
    from concourse.tile import TileContext

    dt = mybir.dt
    f32, bf16 = dt.float32, dt.bfloat16
    AF = mybir.ActivationFunctionType
    AX = mybir.AxisListType
    OP = mybir.AluOpType
    DR = mybir.MatmulPerfMode.DoubleRow

    nc = bacc.Bacc("TRN2", target_bir_lowering=False, debug=False)

    def inp(name, shape, dtype):
        return nc.declare_dram_parameter(name, list(shape), dtype, isOutput=False)

    pA = inp("pA", (WAVES, 128, GPW * PCHUNK * 500), dt.float8e4)
    pxp = inp("pxp", (128, GPC * 128), dt.float8e4)
    bblob = inp("bblob", (128, BCOLS), bf16)
    vxg_d = inp("vxg", (66, VN), bf16)
    vvgc = inp("vvgc", (50, 256), bf16)
    vones = inp("vones", (1, VN), bf16)
    sblob = inp("sblob", (128, 2), f32)
    out_p = nc.declare_dram_parameter("out", [1, VN], f32, isOutput=True)

    with TileContext(nc) as tc:
        with (
            tc.tile_pool(name="const", bufs=1) as cp,
            tc.tile_pool(name="pa", bufs=3) as pap,
            tc.tile_pool(name="ya", bufs=4) as yap,
            tc.tile_pool(name="scrV", bufs=3) as scrV,
            tc.tile_pool(name="big", bufs=1) as bp,
            tc.tile_pool(name="psY", bufs=3, space="PSUM") as psY,
            tc.tile_pool(name="psQ", bufs=3, space="PSUM") as psQ,
            tc.tile_pool(name="psV", bufs=2, space="PSUM") as psV,
        ):
            bb = cp.tile([128, BCOLS], bf16, tag="bblob", name="bb")
            nc.sync.dma_start(out=bb[:, 0:BSPLIT], in_=bblob[:, 0:BSPLIT])
            pxp_t = cp.tile([128, GPC * 128], dt.float8e4, tag="pxp", name="pxp")
            nc.sync.dma_start(out=pxp_t[:], in_=pxp[:])
            vxg = cp.tile([66, VN], bf16, tag="vxg", name="vxg")
            nc.sync.dma_start(out=vxg[:], in_=vxg_d[:])
            vvg = cp.tile([66, 256], bf16, tag="vvg", name="vvg")
            nc.sync.dma_start(out=vvg[16:66, :], in_=vvgc[:])
            sb = cp.tile([128, 2], f32, tag="sblob", name="sb")
            nc.sync.dma_start(out=sb[:], in_=sblob[:])
            nc.sync.dma_start(out=bb[:, BSPLIT:BCOLS], in_=bblob[:, BSPLIT:BCOLS])

            def bslc(name):
                P, F, off = _BSPEC[name]
                return bb[0:P, off:off + F]

            vxt_t, avt_t = bslc("vxt"), bslc("avt")
            w01v_t, w01_t = bslc("w01v"), bslc("w01")
            A1_t, C1_t, D1_t = bslc("A1"), bslc("C1"), bslc("D1")
            hw2_t, hw3_t = bslc("hw2"), bslc("hw3")

            Z = bp.tile([128, GPC], f32, tag="Z")
            Sv = bp.tile([128, GPC], f32, tag="Sv")
            Zb = bp.tile([128, GPC], bf16, tag="Zb")
            Svb = bp.tile([128, GPC], bf16, tag="Svb")
            yav = bp.tile([18, VN], bf16, tag="yav")
            nc.sync.dma_start(out=yav[17:18, :], in_=vones[:])
            h1 = bp.tile([128, VN], bf16, tag="h1")
            av1 = bp.tile([128, VN], bf16, tag="av1")
            xh0 = bp.tile([128, VN], bf16, tag="xh0")
            xh1 = bp.tile([128, VN], bf16, tag="xh1")
            hm = bp.tile([128, VN], bf16, tag="hm")
            ob = bp.tile([1, VN], f32, tag="ob")

            # ---------------- v encoder ----------------
            def v_encoder():
                for qb in range(4):
                    yvt = psV.tile([17, 512], f32, tag="psv", name=f"yv{qb}")
                    for j in range(4):
                        pb = qb * 4 + j
                        nc.tensor.matmul(
                            out=yvt[:, j * 128:(j + 1) * 128],
                            lhsT=vxt_t[:, pb * 17:(pb + 1) * 17],
                            rhs=avt_t[:, pb * 128:(pb + 1) * 128],
                            start=True, stop=True,
                        )
                    nc.vector.tensor_copy(
                        out=yav[0:17, qb * 512:(qb + 1) * 512], in_=yvt[:])
                for qb in range(4):
                    qvt = psV.tile([128, 512], f32, tag="psv", name=f"qv{qb}")
                    for j in range(4):
                        ch = qb * 4 + j
                        nc.tensor.matmul(
                            out=qvt[:, j * 128:(j + 1) * 128],
                            lhsT=yav[:, ch * 128:(ch + 1) * 128],
                            rhs=w01v_t[:],
                            start=True, stop=True,
                        )
                    nc.scalar.activation(
                        out=h1[:, qb * 512:(qb + 1) * 512], in_=qvt[:], func=AF.Relu)
                for qb in range(4):
                    apt = psV.tile([128, 512], f32, tag="psv", name=f"ap{qb}")
                    for j in range(4):
                        pb = qb * 4 + j
                        nc.tensor.matmul(
                            out=apt[:, j * 128:(j + 1) * 128],
                            lhsT=h1[:, pb * 128:(pb + 1) * 128],
                            rhs=avt_t[:, pb * 128:(pb + 1) * 128],
                            start=True, stop=True,
                        )
                    if qb % 2 == 0:
                        nc.vector.tensor_copy(
                            out=av1[:, qb * 512:(qb + 1) * 512], in_=apt[:])
                    else:
                        nc.scalar.activation(
                            out=av1[:, qb * 512:(qb + 1) * 512], in_=apt[:],
                            func=AF.Copy)
                nc.vector.tensor_reduce(
                    out=Sv[:],
                    in_=av1[:].rearrange("p (g n) -> p g n", n=NVP),
                    axis=AX.X, op=OP.add,
                )

            # ---------------- p waves ----------------
            def p_mm1_graph(wv, gj):
                """DR adjacency matmuls for graph wv*4+gj -> yt [18, 512]."""
                pa_t = _pa_tiles[wv]
                g = wv * GPW + gj
                yt = psY.tile([18, 512], f32, tag="yt", name=f"yt{g}")
                for p in range(2):
                    j = gj * PCHUNK + p * 2
                    lhs3 = pxp_t[:, g * 128 + p * 64: g * 128 + (p + 1) * 64]
                    lhs3 = lhs3.rearrange("p (two m) -> p two m", two=2)[:, :, 0:18]
                    rhs3 = pa_t[:, j * 500:(j + 2) * 500]
                    rhs3 = rhs3.rearrange("p (two n) -> p two n", two=2)
                    nc.tensor.matmul(
                        out=yt[:, 0:500], lhsT=lhs3, rhs=rhs3,
                        start=(p == 0), stop=(p == 1), perf_mode=DR,
                    )
                return yt

            def p_drain_graph(g, yt):
                ya = yap.tile([18, 512], bf16, tag="ya", name=f"ya{g}")
                nc.scalar.activation(out=ya[:], in_=yt[:], func=AF.Copy)
                qt = psQ.tile([128, 500], f32, tag="qt", name=f"qt{g}")
                nc.tensor.matmul(
                    out=qt[:], lhsT=w01_t[:], rhs=ya[:, 0:500],
                    start=True, stop=True,
                )
                scr = scrV.tile([128, 500], bf16, tag="scrV", name=f"sV{g}")
                nc.vector.tensor_scalar(
                    out=scr[:], in0=qt[:], scalar1=0.0, scalar2=None,
                    op0=OP.max, op1=OP.add, accum_out=Z[:, g:g + 1])

            # ---------------- head (two 16-graph halves, granular) ----------------
            def head_gg(h):
                base = h * 32
                ggt = psV.tile([48, 256], f32, tag="psv", name=f"gg{h}")
                gsl = slice(base, base + 16)
                hsl = slice(h * 16, (h + 1) * 16)
                nc.vector.tensor_copy(out=Svb[:, hsl], in_=Sv[:, hsl])
                nc.vector.tensor_copy(out=Zb[:, hsl], in_=Z[:, hsl])
                nc.tensor.matmul(
                    out=ggt[gsl, :], lhsT=Svb[:, hsl],
                    rhs=C1_t[:], start=True, stop=False)
                nc.tensor.matmul(
                    out=ggt[gsl, :], lhsT=Zb[:, hsl],
                    rhs=D1_t[:], start=False, stop=True)
                nc.vector.tensor_copy(out=vvg[gsl, :], in_=ggt[gsl, :])

            def head_xps(h, blk, nb):
                xh = (xh0, xh1)[blk]
                bs = slice(blk * 128, (blk + 1) * 128)
                s = slice(h * 1024 + nb * 512, h * 1024 + (nb + 1) * 512)
                xt = psV.tile([128, 512], f32, tag="psv", name=f"xt{h}{blk}{nb}")
                nc.tensor.matmul(out=xt[:], lhsT=A1_t[:, bs],
                                 rhs=av1[:, s], start=True, stop=False)
                nc.tensor.matmul(out=xt[:], lhsT=vvg[:, bs],
                                 rhs=vxg[:, s], start=False, stop=True)
                nc.scalar.activation(
                    out=xh[:, s], in_=xt[:], func=AF.Lrelu, alpha=0.01)

            def head_hm(h, nb):
                s = slice(h * 1024 + nb * 512, h * 1024 + (nb + 1) * 512)
                ht = psV.tile([128, 512], f32, tag="psv", name=f"ht{h}{nb}")
                nc.tensor.matmul(out=ht[:], lhsT=hw2_t[:, 0:128],
                                 rhs=xh0[:, s], start=True, stop=False)
                nc.tensor.matmul(out=ht[:], lhsT=hw2_t[:, 128:256],
                                 rhs=xh1[:, s], start=False, stop=True)
                nc.scalar.activation(
                    out=hm[:, s], in_=ht[:], func=AF.Lrelu,
                    bias=sb[:, 0:1], alpha=0.01)

            def head_ob(h, nb):
                s = slice(h * 1024 + nb * 512, h * 1024 + (nb + 1) * 512)
                lt = psV.tile([1, 512], f32, tag="psv", name=f"lt{h}{nb}")
                nc.tensor.matmul(out=lt[:], lhsT=hw3_t[:], rhs=hm[:, s],
                                 start=True, stop=True)
                nc.scalar.activation(
                    out=ob[:, s], in_=lt[:], func=AF.Identity,
                    bias=sb[0:1, 1:2])
                if nb == 1:
                    osl = slice(h * 1024, (h + 1) * 1024)
                    nc.sync.dma_start(out=out_p[:, osl], in_=ob[:, osl])

            # ---------------- schedule ----------------
            _pa_tiles = {}

            def start_wave(wv):
                pa_t = pap.tile([128, GPW * PCHUNK * 500], dt.float8e4,
                                tag="pa", name=f"pa{wv}")
                nc.sync.dma_start(out=pa_t[:], in_=pA[wv])
                _pa_tiles[wv] = pa_t

            start_wave(0)
            start_wave(1)
            v_encoder()
            # per-graph software pipeline; drains lag mm1 by 2 graphs.
            # head half 0 (graphs 0-15) interleaves with waves 5-6.
            head_sched = {
                18: lambda: head_gg(0),
                19: lambda: head_xps(0, 0, 0), 20: lambda: head_xps(0, 0, 1),
                21: lambda: head_xps(0, 1, 0), 22: lambda: head_xps(0, 1, 1),
                23: lambda: head_hm(0, 0), 24: lambda: head_hm(0, 1),
                25: lambda: head_ob(0, 0), 26: lambda: head_ob(0, 1),
            }
            LAG = 2
            pending = {}
            for t in range(GPC):
                wv, gj = divmod(t, GPW)
                if gj == 0 and wv + 2 < WAVES:
                    start_wave(wv + 2)
                pending[t] = p_mm1_graph(wv, gj)
                if t - LAG in pending:
                    p_drain_graph(t - LAG, pending.pop(t - LAG))
                if t in head_sched:
                    head_sched[t]()
            for t in sorted(pending):
                p_drain_graph(t, pending.pop(t))
            head_gg(1)
            head_xps(1, 0, 0)
            head_xps(1, 0, 1)
            head_xps(1, 1, 0)
            head_xps(1, 1, 1)
            head_hm(1, 0)
            head_hm(1, 1)
            head_ob(1, 0)
            head_ob(1, 1)

    nc.compile()
    return nc


def _host_prep(inp):
    f32 = np.float32
    px = np.asarray(inp["p_x"], f32)
    vx = np.asarray(inp["v_x"], f32)
    pei = np.asarray(inp["p_edge_index"]).astype(np.int64)
    vei = np.asarray(inp["v_edge_index"]).astype(np.int64)
    g = {k: np.asarray(inp[k], f32) for k in
         ("pW0", "pb0", "pW1", "pb1", "pW2", "pb2",
          "vW0", "vb0", "vW1", "vb1", "vW2", "vb2",
          "hW1", "hb1", "hW2", "hb2", "hW3", "hb3")}

    # ---- p-side adjacency (pool weights + fake bias row folded) ----
    psrc, pdst = pei[0], pei[1]
    pdeg = 1.0 + np.bincount(pdst, minlength=B * NP).astype(f32)
    pdinv = (1.0 / np.sqrt(pdeg)).astype(f32)
    csum = pdinv * np.bincount(psrc, weights=pdinv[pdst], minlength=B * NP).astype(f32)
    cp = (csum + pdinv * pdinv) / NP
    AcT = np.zeros((B, 512, 500), f32)
    w = (pdinv[psrc] * pdinv[pdst] * cp[pdst]).astype(f32)
    np.add.at(AcT, (pdst // NP, psrc % NP, pdst % NP), w)
    ar = np.arange(B * NP)
    AcT[ar // NP, ar % NP, ar % NP] += pdinv * pdinv * cp
    AcT[:, 500, :] = cp.reshape(B, NP)
    pa = (np.ascontiguousarray(
        AcT.reshape(NC, WAVES, GPW, PCHUNK, 128, 500).transpose(0, 1, 4, 2, 3, 5)
    ).reshape(NC, WAVES, 128, GPW * PCHUNK * 500) * 256.0).astype(float8_e4m3)

    pxa = np.zeros((B, 512, 18), f32)
    pxa[:, :NP, :16] = px.reshape(B, NP, 16)
    pxa[:, :NP, 16] = 1.0
    pxa[:, 500, 17] = 1.0
    # [core, 128row, graph, pair, plane, 32col]
    px6 = pxa.reshape(NC, GPC, PCHUNK, 128, 18).transpose(0, 3, 1, 2, 4)
    pxp = np.zeros((NC, 128, GPC, 2, 2, 32), f32)
    pxp[..., 0:18] = px6.reshape(NC, 128, GPC, 2, 2, 18)
    pxp = pxp.reshape(NC, 128, GPC * 128).astype(float8_e4m3)

    # ---- v-side adjacency (padded to 64/graph, pairs of graphs) ----
    vsrc, vdst = vei[0], vei[1]
    vdeg = 1.0 + np.bincount(vdst, minlength=B * NV).astype(f32)
    vdinv = (1.0 / np.sqrt(vdeg)).astype(f32)
    AvT = np.zeros((B, NVP, NVP), f32)
    wv_ = (vdinv[vsrc] * vdinv[vdst]).astype(f32)
    np.add.at(AvT, (vdst // NV, vsrc % NV, vdst % NV), wv_)
    arv = np.arange(B * NV)
    AvT[arv // NV, arv % NV, arv % NV] += vdinv * vdinv
    avt_pair = np.zeros((B // 2, 128, 128), f32)
    avt_pair[:, :NVP, :NVP] = AvT[0::2]
    avt_pair[:, NVP:, NVP:] = AvT[1::2]
    avt = np.ascontiguousarray(
        avt_pair.reshape(NC, 16, 128, 128).transpose(0, 2, 1, 3)
    ).reshape(NC, 128, 16 * 128).astype(bfloat16)

    vxa = np.zeros((B, NVP, 17), f32)
    vxa[:, :NV, :16] = vx.reshape(B, NV, 16)
    vxa[:, :NV, 16] = 1.0
    vxt = np.ascontiguousarray(
        vxa.reshape(NC, 16, 128, 17).transpose(0, 2, 1, 3)
    ).reshape(NC, 128, 16 * 17).astype(bfloat16)
    vxTa = np.ascontiguousarray(
        vxa.reshape(NC, VN, 17).transpose(0, 2, 1)
    ).astype(bfloat16)

    # ---- weights + head folds ----
    w01 = np.concatenate(
        [g["pW0"] @ g["pW1"], (g["pb0"] @ g["pW1"])[None], g["pb1"][None]], 0
    ).astype(f32) / 256.0
    w01v = np.concatenate(
        [g["vW0"] @ g["vW1"], (g["vb0"] @ g["vW1"])[None], g["vb1"][None]], 0)
    hW1, hb1 = g["hW1"], g["hb1"]
    ha1o, hbmo = hW1[0:128], hW1[128:256]
    hc1o, hd1o = hW1[256:384], hW1[384:512]
    w0bv = np.concatenate([g["vW0"], g["vb0"][None]], 0)
    A1 = g["vW2"] @ ha1o
    B1 = w0bv @ hbmo
    C1 = g["vW2"] @ hc1o / NV
    D1 = g["pW2"] @ hd1o
    hb1p = hb1 + g["vb2"] @ (ha1o + hc1o) + g["pb2"] @ hd1o
    hw2c = np.ascontiguousarray(
        g["hW2"].reshape(2, 128, 128).transpose(1, 0, 2)).reshape(128, 256)

    gexp = np.zeros((GPC, VN), f32)
    for gi in range(GPC):
        gexp[gi, gi * NVP:(gi + 1) * NVP] = 1.0

    bconsts = {"w01v": w01v, "w01": w01, "A1": A1, "C1": C1, "D1": D1,
               "hw2": hw2c, "hw3": g["hW3"]}
    # vvg rows 16-65: [16 zero rows][16 zero rows (gg half1 overwrites)][B1][hb1p]
    vvgc = np.concatenate(
        [np.zeros((32, 256), f32), B1, hb1p[None]], 0).astype(bfloat16)  # [50, 256]
    sblob = np.zeros((128, 2), f32)
    sblob[:, 0] = g["hb2"]
    sblob[0, 1] = g["hb3"][0]

    in_maps = []
    for c in range(NC):
        bblob = np.zeros((128, BCOLS), bfloat16)
        for name, arr in {**bconsts, "avt": avt[c], "vxt": vxt[c]}.items():
            P, F, off = _BSPEC[name]
            bblob[0:P, off:off + F] = arr.astype(bfloat16)
        vxg = np.zeros((66, VN), bfloat16)
        vxg[0:16] = gexp[0:16].astype(bfloat16)
        vxg[32:48] = gexp[16:32].astype(bfloat16)
        vxg[48:65] = vxTa[c]
        vxg[65] = 1.0
        in_maps.append({
            "pA": pa[c], "pxp": pxp[c], "bblob": bblob,
            "vxg": vxg, "vvgc": vvgc, "sblob": sblob,
            "vones": np.ones((1, VN), bfloat16),
        })
    return in_maps


def _ensure_ntff_hook():
    """Provide antenv.axon_hooks if the image lacks it, so trace=True works."""
    try:
        from antenv.axon_hooks import get_axon_ntff_profile_hook  # noqa: F401
        return
    except ImportError:
        pass
    try:
        import sys
        import types
        import antenv
        from trn_agent_boot.trn_boot import _ntff_profile_via_ctypes

        hook = _ntff_profile_via_ctypes("/opt/axon/libaxon_pjrt.so")
        mod = types.ModuleType("antenv.axon_hooks")
        mod._hook = hook
        mod.get_axon_ntff_profile_hook = lambda: mod._hook
        mod.set_axon_ntff_profile_hook = lambda h: setattr(mod, "_hook", h)
        sys.modules["antenv.axon_hooks"] = mod
        antenv.axon_hooks = mod
    except Exception:
        pass


def kernel(**inputs):
    global _nc_cache, LAST_RESULTS
    from concourse.bass_utils import run_bass_kernel_spmd

    in_maps = _host_prep(inputs)
    if _nc_cache is None:
        _nc_cache = _build_nc()
    trace = os.environ.get("KERNEL_TRACE", "0") == "1"
    if trace:
        _ensure_ntff_hook()
    res = run_bass_kernel_spmd(_nc_cache, in_maps, core_ids=list(range(NC)),
                               trace=trace)
    LAST_RESULTS = res
    outs = [res.results[c]["out"].reshape(GPC, NVP)[:, :NV] for c in range(NC)]
    return np.concatenate(outs, 0).astype(np.float32)
